# revision 20
# baseline (speedup 1.0000x reference)
"""Trainium2 Bass kernel for DressedQuantumCircuit (12 qubits, 6 layers).

Strategy (fp8 DoubleRow formulation):
  - Layer-1 RY gates fold into the input angles; the rest of the circuit is
    a fixed orthogonal U' in R^{4096x4096} precomputed on host from
    `weights`.  Readout contracts to out[b] = sum_j g_j (U'^T s0_b)_j^2
    + post_b, with g_j = sum_w post_w[w] (1-2 bit_w(j)).
  - sqrt|g_j| is folded into U' columns; columns are permuted so the first
    2048 have g>0 and the last 2048 g<0 (exactly balanced by j -> ~j
    symmetry), so the readout is (+1/-1) partition reductions.
  - Data-parallel over batch: 8192 samples -> 8 cores x 1024.
  - Per core: transpose x, pre-matmul -> quarter angles, ACT Sin + two
    double-angle steps -> per-wire (cos G, sin G); half-statevector
    Kronecker factors a[64] (wires 0-5) and blo[64] (wires 6-11) built via
    PE selection matmuls + elementwise product trees; s0[i,b] assembled as
    a replicated-row product, quantized to fp8 e4m3 as hi + exact-residual
    lo (two buffers -> effectively ~12-bit statevector).
  - Main contraction in fp8 with MatmulPerfMode.DoubleRow (256-deep
    contraction per instruction, 0.5 cycles/row): chains H^T s_hi +
    H^T s_lo (+ optional e5m2 residual-of-U chain, L_KP k-pairs).
  - Epilogue: ACT Square, +/- accumulate, (+1/-1)-ones matmul partition
    reduction, final scale + post_b bias.
"""

import hashlib

import numpy as np

N_QUBITS = 12
N_LAYERS = 6
D = 4096                 # 2**N_QUBITS
D_IN = 512
BATCH = 8192
N_CORES = 8
B_CORE = BATCH // N_CORES      # 1024

S_U = 64.0               # fp8 scale for U'
S_S = 16.0               # fp8 scale for s0 (folded into sel wire-11 block)
L_KP = 16                # ut e5m2 residual chain coverage in k-pairs (0..16)

_prog_cache = {}
_ut_cache = {}
_const_cache = {}


# ----------------------------------------------------------------- host math
def _build_ut(weights):
    """Simulate CNOT-chain + layers 2..6 on the identity.  Row i of the
    result is circuit(e_i), i.e. result = U'^T: psi = ut.T @ s0."""
    key = hashlib.sha256(np.ascontiguousarray(weights)).hexdigest()
    if key in _ut_cache:
        return _ut_cache[key]
    N = N_QUBITS
    st = np.eye(D, dtype=np.float32)

    def ry_layer(st, thetas):
        for w in range(N):
            c = np.float32(np.cos(thetas[w] / 2))
            s = np.float32(np.sin(thetas[w] / 2))
            lo = 2 ** (N - 1 - w)
            sh = st.reshape(D, -1, 2, lo)
            a = sh[:, :, 0, :].copy()
            b = sh[:, :, 1, :]
            sh[:, :, 0, :] = c * a - s * b
            sh[:, :, 1, :] = s * a + c * b
        return st

    def cnot_chain(st):
        for w in range(N - 1):
            lt = 2 ** (N - 2 - w)
            sh = st.reshape(D, -1, 2, 1, 2, lt)
            a = sh[:, :, 1, :, 0, :].copy()
            sh[:, :, 1, :, 0, :] = sh[:, :, 1, :, 1, :]
            sh[:, :, 1, :, 1, :] = a
        return st

    wts = np.asarray(weights, dtype=np.float64)
    st = cnot_chain(st)
    for layer in range(1, N_LAYERS):
        st = ry_layer(st, wts[layer])
        st = cnot_chain(st)
    _ut_cache.clear()
    _ut_cache[key] = st
    return st


def _host_constants(pre_w, pre_b, weights, post_w, post_b):
    import ml_dtypes

    key = hashlib.sha256(
        b"".join(np.ascontiguousarray(np.asarray(a, np.float64))
                 for a in (pre_w, pre_b, weights, post_w, post_b))
    ).hexdigest() + f"_{L_KP}"
    if key in _const_cache:
        return _const_cache[key]

    N = N_QUBITS
    wts = np.asarray(weights, dtype=np.float64)
    # quarter angle: a = x @ (pre_w.T * pi/16) + bias_a, full angle G = 4a,
    # v0 = cos(G), v1 = sin(G)
    wf = (np.asarray(pre_w, dtype=np.float64).T * (np.pi / 16.0))  # [512, 12]
    bias_a = (np.asarray(pre_b, dtype=np.float64) * (np.pi / 4.0)
              + wts[0] / 2.0 + np.pi / 4.0) / 4.0
    wf_packed = np.ascontiguousarray(
        wf.reshape(4, 128, N).transpose(1, 0, 2).reshape(128, 4 * N)
    ).astype(np.float32)
    bias_sa = bias_a.reshape(N, 1).astype(np.float32)
    bias_ca = (bias_a + np.pi / 2.0).reshape(N, 1).astype(np.float32)

    # readout vector g, sign-sorted column permutation
    j = np.arange(D)
    g = np.zeros(D, dtype=np.float64)
    for w in range(N):
        g += float(np.asarray(post_w).reshape(-1)[w]) * (
            1.0 - 2.0 * ((j >> (N - 1 - w)) & 1))
    pos = np.where(g > 0)[0]
    neg = np.where(g <= 0)[0]
    assert len(pos) == D // 2 and len(neg) == D // 2, (len(pos), len(neg))
    perm = np.concatenate([pos, neg])
    sg = np.sqrt(np.abs(g[perm])).astype(np.float32)

    ut = _build_ut(weights)                       # [4096 i, 4096 j]
    utg = ut[:, perm].astype(np.float32) * sg[None, :]
    Hq = (utg * np.float32(S_U)).astype(ml_dtypes.float8_e4m3)
    H32 = Hq.astype(np.float32)
    Rq = (utg * np.float32(S_U) - H32).astype(ml_dtypes.float8_e5m2)

    def pack(M):
        # [4096 k, 4096 j] -> [32 j, 128 p, 16 kp, 2 i, 128 jj]
        # k = kp*256 + i*128 + p
        return np.ascontiguousarray(
            M.reshape(16, 2, 128, 32, 128).transpose(3, 2, 0, 1, 4))

    ut_h = pack(Hq)
    ut_l = pack(Rq)[:, :, :max(L_KP, 1), :, :].copy()

    # sel [44, 12*64]: F_w[q, b] = v_{w, bit}(b); v0 rows 0..11, v1 rows
    # 32..43 (32-aligned for DVE writes); wire-11 block carries S_S.
    sel = np.zeros((44, 12 * 64), np.float32)
    q64 = np.arange(64)
    for w in range(12):
        bit = (q64 >> (5 - (w % 6))) & 1
        sel[w, w * 64 + q64[bit == 0]] = 1.0
        sel[32 + w, w * 64 + q64[bit == 1]] = 1.0
    sel[:, 11 * 64:12 * 64] *= np.float32(S_S)

    # rsel [64, 33*128]: blocks t=0..31: rep_t[p,b] = aT[2t + (p>>6), b];
    # block 32: bloTrep[p,b] = bloT[p & 63, b]
    rsel = np.zeros((64, 33 * 128), np.float32)
    p128 = np.arange(128)
    for t in range(32):
        rsel[2 * t + (p128 >> 6), t * 128 + p128] = 1.0
    rsel[(p128 & 63), 32 * 128 + p128] = 1.0

    pb = np.asarray(post_b, dtype=np.float32).reshape(1, 1)
    out = (wf_packed, bias_sa, bias_ca, sel, rsel, ut_h, ut_l, pb)
    _const_cache.clear()
    _const_cache[key] = out
    return out


# ------------------------------------------------------------- device program
def _build_program(reps=1, l_kp=L_KP):
    import concourse.bass as bass
    import concourse.mybir as mybir
    import concourse.tile as tile
    from concourse import bacc
    from concourse.masks import make_identity

    f32 = mybir.dt.float32
    f32r = mybir.dt.float32r
    e4 = mybir.dt.float8e4
    bf16 = mybir.dt.bfloat16
    e5 = mybir.dt.float8e5
    AF = mybir.ActivationFunctionType
    ALU = mybir.AluOpType
    DR = mybir.MatmulPerfMode.DoubleRow
    N = N_QUBITS

    nc = bacc.Bacc("TRN2", target_bir_lowering=False, debug=False,
                   num_devices=N_CORES)
    x_d = nc.dram_tensor("x", [B_CORE, D_IN], f32r, kind="ExternalInput").ap()
    wf_d = nc.dram_tensor("wf", [128, 4 * N], f32r, kind="ExternalInput").ap()
    bsa_d = nc.dram_tensor("bsa", [N, 1], f32, kind="ExternalInput").ap()
    bca_d = nc.dram_tensor("bca", [N, 1], f32, kind="ExternalInput").ap()
    sel_d = nc.dram_tensor("sel", [44, 12 * 64], f32r, kind="ExternalInput").ap()
    rsel_d = nc.dram_tensor("rsel", [64, 33 * 128], f32r,
                            kind="ExternalInput").ap()
    uth_d = nc.dram_tensor("uth", [32, 128, 16, 2, 128], e4,
                           kind="ExternalInput").ap()
    utl_d = nc.dram_tensor("utl", [32, 128, max(l_kp, 1), 2, 128], e5,
                           kind="ExternalInput").ap()
    pb_d = nc.dram_tensor("pb", [1, 1], f32, kind="ExternalInput").ap()
    out_d = nc.dram_tensor("out", [1, B_CORE], f32, kind="ExternalOutput").ap()

    with tile.TileContext(nc) as tc:
        with (
            tc.tile_pool(name="const", bufs=1) as constp,
            tc.tile_pool(name="ab", bufs=1) as abp,
            tc.tile_pool(name="s0", bufs=1) as s0p,
            tc.tile_pool(name="tmp", bufs=2) as tmpp,
            tc.tile_pool(name="ut", bufs=3) as utp,
            tc.tile_pool(name="acc", bufs=1) as accp,
            tc.tile_pool(name="sq", bufs=1) as sqp,
        ):
            wf_sb = constp.tile([128, 4 * N], f32r)
            nc.gpsimd.dma_start(wf_sb[:], wf_d[:])
            bsa = constp.tile([N, 1], f32)
            nc.gpsimd.dma_start(bsa[:], bsa_d[:])
            bca = constp.tile([N, 1], f32)
            nc.gpsimd.dma_start(bca[:], bca_d[:])
            sel_sb = constp.tile([44, 12 * 64], f32r)
            nc.scalar.dma_start(sel_sb[:], sel_d[:])
            rsel_sb = constp.tile([64, 33 * 128], f32r)
            nc.scalar.dma_start(rsel_sb[:], rsel_d[:])
            pb_sb = constp.tile([1, 1], f32)
            nc.gpsimd.dma_start(pb_sb[:], pb_d[:])
            pm0 = constp.tile([128, 2], f32)
            nc.gpsimd.memset(pm0[:, 0:1], 1.0)
            nc.gpsimd.memset(pm0[:, 1:2], -1.0)
            pm1 = constp.tile([128, 2], f32r)
            nc.vector.tensor_copy(pm1[:], pm0[:])
            ones = pm1[:, 0:1]
            mones = pm1[:, 1:2]

            for rep in range(reps):
                sfx = f"_{rep}"
                aT = abp.tile([64, B_CORE], f32r, tag="aT", name="aT" + sfx)
                bloT = abp.tile([64, B_CORE], f32r, tag="bloT",
                                name="bloT" + sfx)
                bloTrep = abp.tile([128, B_CORE], f32r, tag="bTr",
                                   name="bTr" + sfx)
                # ---------- prologue: x -> angles -> (cos G, sin G) -> a, blo
                with tc.tile_pool(name="pro", bufs=1) as prop:
                    ident0 = prop.tile([128, 128], f32, tag="id0",
                                       name="ident0" + sfx)
                    make_identity(nc, ident0[:])
                    ident = prop.tile([128, 128], f32r, tag="ident",
                                      name="ident" + sfx)
                    nc.vector.tensor_copy(ident[:], ident0[:])
                    xT = prop.tile([128, 4, B_CORE], f32r, tag="xT",
                                   name="xT" + sfx)
                    vboth = prop.tile([44, B_CORE], f32r, tag="vb",
                                      name="vb" + sfx)
                    nc.scalar.activation(vboth[0:32, :],
                                         rsel_sb[0:32, 0:B_CORE],
                                         AF.Copy, scale=0.0)
                    with (
                        tc.tile_pool(name="trps", bufs=2,
                                     space="PSUM") as tr_ps,
                        tc.tile_pool(name="preps", bufs=1,
                                     space="PSUM") as pre_ps,
                    ):
                        for cc in range(8):
                            xt = prop.tile([128, D_IN], f32r,
                                           tag=f"xt{cc % 2}",
                                           name=f"xt{cc}{sfx}")
                            (nc.sync if cc % 2 == 0 else nc.gpsimd
                             ).dma_start(
                                xt[:], x_d[cc * 128:(cc + 1) * 128, :])
                            tp = tr_ps.tile([128, 4, 128], f32r, tag="tp")
                            for kf in range(4):
                                nc.tensor.transpose(
                                    tp[:, kf, :],
                                    xt[:, kf * 128:(kf + 1) * 128], ident[:])
                            dst = xT[:, :, cc * 128:(cc + 1) * 128]
                            if cc % 2:
                                nc.vector.tensor_copy(dst, tp[:])
                            else:
                                nc.scalar.activation(dst, tp[:], AF.Copy)
                        # pre-matmul: quarter angles [12, 1024]
                        pre = pre_ps.tile([N, B_CORE], f32, tag="pre")
                        for h in (0, 1):
                            hs = slice(h * 512, (h + 1) * 512)
                            for kf in range(4):
                                nc.tensor.matmul(
                                    pre[:, hs],
                                    wf_sb[:, kf * N:(kf + 1) * N]
                                    ,
                                    xT[:, kf, hs],
                                    start=(kf == 0), stop=(kf == 3))
                        # sin/cos of quarter angle + two double-angle steps
                        sa = prop.tile([N, B_CORE], f32, tag="v0",
                                       name="sa" + sfx)
                        ca = prop.tile([N, B_CORE], f32, tag="v1",
                                       name="ca" + sfx)
                        nc.scalar.activation(sa[:], pre[:], AF.Sin,
                                             bias=bsa[:])
                        nc.scalar.activation(ca[:], pre[:], AF.Sin,
                                             bias=bca[:])
                        t1 = prop.tile([N, B_CORE], f32, tag="v2",
                                       name="t1" + sfx)
                        u1 = prop.tile([N, B_CORE], f32, tag="v3",
                                       name="u1" + sfx)
                        nc.vector.tensor_mul(t1[:], sa[:], ca[:])
                        nc.vector.tensor_mul(u1[:], sa[:], sa[:])
                        c2 = prop.tile([N, B_CORE], f32, tag="v1",
                                       name="c2" + sfx)
                        nc.scalar.activation(c2[:], u1[:], AF.Copy, bias=1.0,
                                             scale=-2.0)
                        # t2 = (2 t1) c2 = s2 c2 ; u2 = (4 t1) t1 = s2^2
                        t2 = prop.tile([N, B_CORE], f32, tag="v0",
                                       name="t2" + sfx)
                        u2 = prop.tile([N, B_CORE], f32, tag="v3",
                                       name="u2" + sfx)
                        nc.vector.scalar_tensor_tensor(
                            t2[:], t1[:], 2.0, c2[:], ALU.mult, ALU.mult)
                        nc.vector.scalar_tensor_tensor(
                            u2[:], t1[:], 4.0, t1[:], ALU.mult, ALU.mult)
                        # v1 = sin G = 2 t2 (rows 32..43), v0 = cos G = 1-2u2
                        nc.vector.tensor_add(vboth[32:32 + N, :], t2[:],
                                             t2[:])
                        nc.scalar.activation(vboth[0:N, :], u2[:], AF.Copy,
                                             bias=1.0, scale=-2.0)

                    # half-kron factors aT, bloT + bloTrep
                    with tc.tile_pool(name="fps", bufs=1,
                                      space="PSUM") as f_ps:
                        for half, dst in ((0, aT), (1, bloT)):
                            Ms = []
                            for pair in range(3):
                                Fp = [None, None]
                                for k in range(2):
                                    w = half * 6 + pair * 2 + k
                                    Fp[k] = f_ps.tile(
                                        [64, B_CORE], f32, tag=f"F{k}",
                                        name=f"F{k}_{half}{pair}{sfx}")
                                    for h in (0, 1):
                                        hs = slice(h * 512, (h + 1) * 512)
                                        nc.tensor.matmul(
                                            Fp[k][:, hs],
                                            sel_sb[:, w * 64:(w + 1) * 64]
                                            ,
                                            vboth[:, hs],
                                            start=True, stop=True)
                                M = prop.tile(
                                    [64, B_CORE], f32, tag=f"m{pair}",
                                    name=f"M{pair}_{half}{sfx}")
                                fcp = prop.tile(
                                    [64, B_CORE], f32, tag="fcp",
                                    name=f"fcp_{half}{pair}{sfx}")
                                nc.scalar.activation(fcp[:], Fp[1][:],
                                                     AF.Copy)
                                nc.vector.tensor_mul(M[:], Fp[0][:],
                                                     fcp[:])
                                Ms.append(M)
                            M01 = prop.tile([64, B_CORE], f32, tag="fcp",
                                            name=f"M01_{half}{sfx}")
                            nc.gpsimd.tensor_mul(M01[:], Ms[0][:],
                                                 Ms[1][:])
                            nc.gpsimd.tensor_mul(dst[:], M01[:], Ms[2][:])
                        # bloTrep[p, b] = bloT[p & 63, b]
                        bt = f_ps.tile([128, B_CORE], f32, tag="bt")
                        for h in (0, 1):
                            hs = slice(h * 512, (h + 1) * 512)
                            nc.tensor.matmul(
                                bt[:, hs],
                                rsel_sb[:, 32 * 128:33 * 128],
                                bloT[:, hs],
                                start=True, stop=True)
                        nc.scalar.activation(bloTrep[:], bt[:], AF.Copy)

                # ---------- s0 generation, main DR matmul, epilogue
                s0hi = [s0p.tile([128, 2, B_CORE], e4, tag=f"s0h{g_}",
                                 name=f"s0h{g_}{sfx}") for g_ in range(16)]
                s0lo = [s0p.tile([128, 2, B_CORE], e4, tag=f"s0l{g_}",
                                 name=f"s0l{g_}{sfx}") for g_ in range(16)]
                acc_pos = accp.tile([128, B_CORE], f32r, tag="accp",
                                    name="accp" + sfx)
                acc_neg = accp.tile([128, B_CORE], f32r, tag="accn",
                                    name="accn" + sfx)
                out_sb = accp.tile([1, B_CORE], f32, tag="osb",
                                   name="osb" + sfx)
                with (
                    tc.tile_pool(name="repps", bufs=1, space="PSUM") as rep_ps,
                    tc.tile_pool(name="mmps", bufs=2, space="PSUM") as mm_ps,
                    tc.tile_pool(name="finps", bufs=1, space="PSUM") as fin_ps,
                ):
                    # s0 build: rep_t = a-rows replicated; * bloTrep -> fp8
                    # hi + exact fp8 residual lo
                    for t in range(32):
                        g_, i_ = t // 2, t % 2
                        rp = rep_ps.tile([128, B_CORE], f32, tag="rp")
                        for h in (0, 1):
                            hs = slice(h * 512, (h + 1) * 512)
                            nc.tensor.matmul(
                                rp[:, hs],
                                rsel_sb[:, t * 128:(t + 1) * 128]
                                ,
                                aT[:, hs],
                                start=True, stop=True)
                        pool_t = t % 8 < 5
                        seng = nc.gpsimd if pool_t else nc.vector
                        tmp = tmpp.tile([128, B_CORE], bf16,
                                        tag=f"tmp{int(pool_t)}")
                        nc.vector.tensor_mul(tmp[:], rp[:], bloTrep[:])
                        nc.scalar.activation(s0hi[g_][:, i_, :], tmp[:],
                                             AF.Copy)
                        seng.tensor_sub(s0lo[g_][:, i_, :], tmp[:],
                                        s0hi[g_][:, i_, :])

                    # main DoubleRow chains
                    for jt in range(32):
                        utt = utp.tile([128, 16, 2, 128], e4, tag="uth")
                        nc.sync.dma_start(utt[:], uth_d[jt])
                        if l_kp:
                            utl = utp.tile([128, l_kp, 2, 128], e5,
                                           tag="utl")
                            nc.gpsimd.dma_start(utl[:], utl_d[jt])
                        sq = sqp.tile([128, B_CORE], f32, tag=f"sq{jt % 2}",
                                      name=f"sq{jt}{sfx}")
                        for h in (0, 1):
                            hs = slice(h * 512, (h + 1) * 512)
                            ps = mm_ps.tile([128, 512], f32, tag=f"mm{h}")
                            for kp in range(16):
                                nc.tensor.matmul(
                                    ps[:], utt[:, kp], s0hi[kp][:, :, hs],
                                    start=(kp == 0), stop=False,
                                    perf_mode=DR)
                            for kp in range(16):
                                last = (l_kp == 0) and (kp == 15)
                                nc.tensor.matmul(
                                    ps[:], utt[:, kp], s0lo[kp][:, :, hs],
                                    start=False, stop=last, perf_mode=DR)
                            for kp in range(l_kp):
                                nc.tensor.matmul(
                                    ps[:], utl[:, kp], s0hi[kp][:, :, hs],
                                    start=False, stop=(kp == l_kp - 1),
                                    perf_mode=DR)
                            nc.scalar.activation(sq[:, hs], ps[:], AF.Square)
                        acc = acc_pos if jt < 16 else acc_neg
                        eng = nc.vector if jt < 16 else nc.gpsimd
                        if jt % 16 == 0:
                            eng.tensor_copy(acc[:], sq[:])
                        else:
                            eng.tensor_add(acc[:], acc[:], sq[:])

                    # final +/- partition reduction, scale, bias; the out
                    # row reuses acc_pos partition 0 (acc dead after fin)
                    for h in (0, 1):
                        hs = slice(h * 512, (h + 1) * 512)
                        fin = fin_ps.tile([1, 512], f32, tag="fin",
                                          name=f"fin{h}{sfx}")
                        nc.tensor.matmul(fin[:], ones,
                                         acc_pos[:, hs],
                                         start=True, stop=False)
                        nc.tensor.matmul(fin[:], mones,
                                         acc_neg[:, hs],
                                         start=False, stop=True)
                        nc.scalar.activation(
                            out_sb[:, hs], fin[:], AF.Identity,
                            bias=pb_sb[:], scale=float(1.0 / (S_U * S_S) ** 2))
                    nc.sync.dma_start(out_d[:], out_sb[:])

    nc.compile()
    return nc


# ------------------------------------------------------------------- entry
def _core_input_map(inputs, core):
    x = np.ascontiguousarray(np.asarray(inputs["x"], dtype=np.float32))
    wf_packed, bias_sa, bias_ca, sel, rsel, ut_h, ut_l, pb = _host_constants(
        inputs["pre_w"], inputs["pre_b"], inputs["weights"],
        inputs["post_w"], inputs["post_b"])
    return {
        "x": x[core * B_CORE:(core + 1) * B_CORE],
        "wf": wf_packed, "bsa": bias_sa, "bca": bias_ca,
        "sel": sel, "rsel": rsel, "uth": ut_h, "utl": ut_l, "pb": pb,
    }


def kernel(x, pre_w, pre_b, weights, post_w, post_b):
    from concourse import bass_utils

    inputs = {"x": x, "pre_w": pre_w, "pre_b": pre_b, "weights": weights,
              "post_w": post_w, "post_b": post_b}

    if "nc" not in _prog_cache:
        _prog_cache["nc"] = _build_program()
    nc = _prog_cache["nc"]

    in_maps = [_core_input_map(inputs, c) for c in range(N_CORES)]
    res = bass_utils.run_bass_kernel_spmd(nc, in_maps,
                                          core_ids=list(range(N_CORES)))
    out = np.concatenate([r["out"][0] for r in res.results])
    return out.reshape(BATCH, 1).astype(np.float32)


# revision 22
# speedup vs baseline: 1.3293x; 1.3293x over previous
"""Trainium2 Bass kernel for DressedQuantumCircuit (12 qubits, 6 layers).

Strategy (fp8 DoubleRow formulation):
  - Layer-1 RY gates fold into the input angles; the rest of the circuit is
    a fixed orthogonal U' in R^{4096x4096} precomputed on host from
    `weights`.  Readout contracts to out[b] = sum_j g_j (U'^T s0_b)_j^2
    + post_b, with g_j = sum_w post_w[w] (1-2 bit_w(j)).
  - sqrt|g_j| is folded into U' columns; columns are permuted so the first
    2048 have g>0 and the last 2048 g<0 (exactly balanced by j -> ~j
    symmetry), so the readout is (+1/-1) partition reductions.
  - Data-parallel over batch: 8192 samples -> 8 cores x 1024.
  - Per core: transpose x, pre-matmul -> quarter angles, ACT Sin + two
    double-angle steps -> per-wire (cos G, sin G); half-statevector
    Kronecker factors a[64] (wires 0-5) and blo[64] (wires 6-11) built via
    PE selection matmuls + elementwise product trees; s0[i,b] assembled as
    a replicated-row product, quantized to fp8 e4m3 as hi + exact-residual
    lo (two buffers -> effectively ~12-bit statevector).
  - Main contraction in fp8 with MatmulPerfMode.DoubleRow (256-deep
    contraction per instruction, 0.5 cycles/row): chains H^T s_hi +
    H^T s_lo (+ optional e5m2 residual-of-U chain, L_KP k-pairs).
  - Epilogue: ACT Square, +/- accumulate, (+1/-1)-ones matmul partition
    reduction, final scale + post_b bias.
"""

import hashlib

import numpy as np

N_QUBITS = 12
N_LAYERS = 6
D = 4096                 # 2**N_QUBITS
D_IN = 512
BATCH = 8192
N_CORES = 8
B_CORE = BATCH // N_CORES      # 1024

S_U = 64.0               # fp8 scale for U'
S_S = 16.0               # fp8 scale for s0 (folded into sel wire-11 block)
L_KP = 0                 # ut e5m2 residual chain coverage in k-pairs (0..16)

_prog_cache = {}
_ut_cache = {}
_const_cache = {}


# ----------------------------------------------------------------- host math
def _build_ut(weights):
    """Simulate CNOT-chain + layers 2..6 on the identity.  Row i of the
    result is circuit(e_i), i.e. result = U'^T: psi = ut.T @ s0."""
    key = hashlib.sha256(np.ascontiguousarray(weights)).hexdigest()
    if key in _ut_cache:
        return _ut_cache[key]
    N = N_QUBITS
    st = np.eye(D, dtype=np.float32)

    def ry_layer(st, thetas):
        for w in range(N):
            c = np.float32(np.cos(thetas[w] / 2))
            s = np.float32(np.sin(thetas[w] / 2))
            lo = 2 ** (N - 1 - w)
            sh = st.reshape(D, -1, 2, lo)
            a = sh[:, :, 0, :].copy()
            b = sh[:, :, 1, :]
            sh[:, :, 0, :] = c * a - s * b
            sh[:, :, 1, :] = s * a + c * b
        return st

    def cnot_chain(st):
        for w in range(N - 1):
            lt = 2 ** (N - 2 - w)
            sh = st.reshape(D, -1, 2, 1, 2, lt)
            a = sh[:, :, 1, :, 0, :].copy()
            sh[:, :, 1, :, 0, :] = sh[:, :, 1, :, 1, :]
            sh[:, :, 1, :, 1, :] = a
        return st

    wts = np.asarray(weights, dtype=np.float64)
    st = cnot_chain(st)
    for layer in range(1, N_LAYERS):
        st = ry_layer(st, wts[layer])
        st = cnot_chain(st)
    _ut_cache.clear()
    _ut_cache[key] = st
    return st


def _host_constants(pre_w, pre_b, weights, post_w, post_b):
    import ml_dtypes

    key = hashlib.sha256(
        b"".join(np.ascontiguousarray(np.asarray(a, np.float64))
                 for a in (pre_w, pre_b, weights, post_w, post_b))
    ).hexdigest() + f"_{L_KP}"
    if key in _const_cache:
        return _const_cache[key]

    N = N_QUBITS
    wts = np.asarray(weights, dtype=np.float64)
    # quarter angle: a = x @ (pre_w.T * pi/16) + bias_a, full angle G = 4a,
    # v0 = cos(G), v1 = sin(G)
    wf = (np.asarray(pre_w, dtype=np.float64).T * (np.pi / 16.0))  # [512, 12]
    bias_a = (np.asarray(pre_b, dtype=np.float64) * (np.pi / 4.0)
              + wts[0] / 2.0 + np.pi / 4.0) / 4.0
    wf_packed = np.ascontiguousarray(
        wf.reshape(4, 128, N).transpose(1, 0, 2).reshape(128, 4 * N)
    ).astype(np.float32)
    bias_sa = bias_a.reshape(N, 1).astype(np.float32)
    bias_ca = (bias_a + np.pi / 2.0).reshape(N, 1).astype(np.float32)

    # readout vector g, sign-sorted column permutation
    j = np.arange(D)
    g = np.zeros(D, dtype=np.float64)
    for w in range(N):
        g += float(np.asarray(post_w).reshape(-1)[w]) * (
            1.0 - 2.0 * ((j >> (N - 1 - w)) & 1))
    pos = np.where(g > 0)[0]
    neg = np.where(g <= 0)[0]
    assert len(pos) == D // 2 and len(neg) == D // 2, (len(pos), len(neg))
    perm = np.concatenate([pos, neg])
    sg = np.sqrt(np.abs(g[perm])).astype(np.float32)

    ut = _build_ut(weights)                       # [4096 i, 4096 j]
    utg = ut[:, perm].astype(np.float32) * sg[None, :]
    Hq = (utg * np.float32(S_U)).astype(ml_dtypes.float8_e4m3)
    H32 = Hq.astype(np.float32)
    Rq = (utg * np.float32(S_U) - H32).astype(ml_dtypes.float8_e5m2)

    def pack(M):
        # [4096 k, 4096 j] -> [32 j, 128 p, 16 kp, 2 i, 128 jj]
        # k = kp*256 + i*128 + p
        return np.ascontiguousarray(
            M.reshape(16, 2, 128, 32, 128).transpose(3, 2, 0, 1, 4))

    ut_h = pack(Hq)
    ut_l = pack(Rq)[:, :, :max(L_KP, 1), :, :].copy()

    # sel [44, 12*64]: F_w[q, b] = v_{w, bit}(b); v0 rows 0..11, v1 rows
    # 32..43 (32-aligned for DVE writes); wire-11 block carries S_S.
    sel = np.zeros((44, 12 * 64), np.float32)
    q64 = np.arange(64)
    for w in range(12):
        bit = (q64 >> (5 - (w % 6))) & 1
        sel[w, w * 64 + q64[bit == 0]] = 1.0
        sel[32 + w, w * 64 + q64[bit == 1]] = 1.0
    sel[:, 11 * 64:12 * 64] *= np.float32(S_S)

    # rsel [64, 33*128]: blocks t=0..31: rep_t[p,b] = aT[2t + (p>>6), b];
    # block 32: bloTrep[p,b] = bloT[p & 63, b]
    rsel = np.zeros((64, 33 * 128), np.float32)
    p128 = np.arange(128)
    for t in range(32):
        rsel[2 * t + (p128 >> 6), t * 128 + p128] = 1.0
    rsel[(p128 & 63), 32 * 128 + p128] = 1.0

    pb = np.asarray(post_b, dtype=np.float32).reshape(1, 1)
    out = (wf_packed, bias_sa, bias_ca, sel, rsel, ut_h, ut_l, pb)
    _const_cache.clear()
    _const_cache[key] = out
    return out


# ------------------------------------------------------------- device program
def _build_program(reps=1, l_kp=L_KP):
    import concourse.bass as bass
    import concourse.mybir as mybir
    import concourse.tile as tile
    from concourse import bacc
    from concourse.masks import make_identity

    f32 = mybir.dt.float32
    f32r = mybir.dt.float32r
    e4 = mybir.dt.float8e4
    bf16 = mybir.dt.bfloat16
    e5 = mybir.dt.float8e5
    AF = mybir.ActivationFunctionType
    ALU = mybir.AluOpType
    DR = mybir.MatmulPerfMode.DoubleRow
    N = N_QUBITS

    nc = bacc.Bacc("TRN2", target_bir_lowering=False, debug=False,
                   num_devices=N_CORES)
    x_d = nc.dram_tensor("x", [B_CORE, D_IN], f32r, kind="ExternalInput").ap()
    wf_d = nc.dram_tensor("wf", [128, 4 * N], f32r, kind="ExternalInput").ap()
    bsa_d = nc.dram_tensor("bsa", [N, 1], f32, kind="ExternalInput").ap()
    bca_d = nc.dram_tensor("bca", [N, 1], f32, kind="ExternalInput").ap()
    sel_d = nc.dram_tensor("sel", [44, 12 * 64], f32r,
                           kind="ExternalInput").ap()
    rsel_d = nc.dram_tensor("rsel", [64, 33 * 128], f32r,
                            kind="ExternalInput").ap()
    uth_d = nc.dram_tensor("uth", [32, 128, 16, 2, 128], e4,
                           kind="ExternalInput").ap()
    utl_d = nc.dram_tensor("utl", [32, 128, max(l_kp, 1), 2, 128], e5,
                           kind="ExternalInput").ap()
    pb_d = nc.dram_tensor("pb", [1, 1], f32, kind="ExternalInput").ap()
    out_d = nc.dram_tensor("out", [1, B_CORE], f32, kind="ExternalOutput").ap()

    HB = 512                       # half-batch width

    with tile.TileContext(nc) as tc:
        with (
            tc.tile_pool(name="const", bufs=1) as constp,
            tc.tile_pool(name="ab", bufs=1) as abp,
            tc.tile_pool(name="s0", bufs=1) as s0p,
            tc.tile_pool(name="tmp", bufs=2) as tmpp,
            tc.tile_pool(name="ut", bufs=3) as utp,
            tc.tile_pool(name="acc", bufs=1) as accp,
            tc.tile_pool(name="sq", bufs=1) as sqp,
        ):
            wf_sb = constp.tile([128, 4 * N], f32r)
            nc.gpsimd.dma_start(wf_sb[:], wf_d[:])
            bsa = constp.tile([N, 1], f32)
            nc.gpsimd.dma_start(bsa[:], bsa_d[:])
            bca = constp.tile([N, 1], f32)
            nc.gpsimd.dma_start(bca[:], bca_d[:])
            sel_sb = constp.tile([44, 12 * 64], f32r)
            nc.scalar.dma_start(sel_sb[:], sel_d[:])
            rsel_sb = constp.tile([64, 33 * 128], f32r)
            nc.scalar.dma_start(rsel_sb[:], rsel_d[:])
            pb_sb = constp.tile([1, 1], f32)
            nc.gpsimd.dma_start(pb_sb[:], pb_d[:])
            pm0 = constp.tile([128, 2], f32)
            nc.gpsimd.memset(pm0[:, 0:1], 1.0)
            nc.gpsimd.memset(pm0[:, 1:2], -1.0)
            pm1 = constp.tile([128, 2], f32r)
            nc.vector.tensor_copy(pm1[:], pm0[:])
            ones = pm1[:, 0:1]
            mones = pm1[:, 1:2]

            for rep in range(reps):
                sfx = f"_{rep}"
                aT = abp.tile([64, B_CORE], f32r, tag="aT", name="aT" + sfx)
                bloT = abp.tile([64, B_CORE], f32r, tag="bloT",
                                name="bloT" + sfx)
                bloTrep = abp.tile([128, B_CORE], f32r, tag="bTr",
                                   name="bTr" + sfx)
                # ---------- prologue: x -> angles -> (cos G, sin G) -> a, blo
                # (everything after the pre-matmul is split by batch half so
                # the h=0 pipeline races ahead)
                with tc.tile_pool(name="pro", bufs=1) as prop:
                    ident0 = prop.tile([128, 128], f32, tag="id0",
                                       name="ident0" + sfx)
                    make_identity(nc, ident0[:])
                    ident = prop.tile([128, 128], f32r, tag="ident",
                                      name="ident" + sfx)
                    nc.vector.tensor_copy(ident[:], ident0[:])
                    xT = prop.tile([128, 4, B_CORE], f32r, tag="xT",
                                   name="xT" + sfx)
                    vboth = prop.tile([44, B_CORE], f32r, tag="vb",
                                      name="vb" + sfx)
                    nc.scalar.activation(vboth[0:32, :],
                                         rsel_sb[0:32, 0:B_CORE],
                                         AF.Copy, scale=0.0)
                    with (
                        tc.tile_pool(name="trps", bufs=2,
                                     space="PSUM") as tr_ps,
                        tc.tile_pool(name="preps", bufs=1,
                                     space="PSUM") as pre_ps,
                    ):
                        for cc in range(8):
                            xt = prop.tile([128, D_IN], f32r,
                                           tag=f"xt{cc % 2}",
                                           name=f"xt{cc}{sfx}")
                            (nc.sync if cc % 2 == 0 else nc.gpsimd
                             ).dma_start(
                                xt[:], x_d[cc * 128:(cc + 1) * 128, :])
                            tp = tr_ps.tile([128, 4, 128], f32r, tag="tp")
                            for kf in range(4):
                                nc.tensor.transpose(
                                    tp[:, kf, :],
                                    xt[:, kf * 128:(kf + 1) * 128], ident[:])
                            dst = xT[:, :, cc * 128:(cc + 1) * 128]
                            if cc % 2:
                                nc.vector.tensor_copy(dst, tp[:])
                            else:
                                nc.scalar.activation(dst, tp[:], AF.Copy)
                        # pre-matmul: quarter angles [12, 1024]
                        pre = pre_ps.tile([N, B_CORE], f32, tag="pre")
                        for h in (0, 1):
                            hs = slice(h * HB, (h + 1) * HB)
                            for kf in range(4):
                                nc.tensor.matmul(
                                    pre[:, hs],
                                    wf_sb[:, kf * N:(kf + 1) * N],
                                    xT[:, kf, hs],
                                    start=(kf == 0), stop=(kf == 3))
                        # per-half double-angle chain
                        for h in (0, 1):
                            hs = slice(h * HB, (h + 1) * HB)
                            sa = prop.tile([N, HB], f32, tag=f"sa{h}",
                                           name=f"sa{h}{sfx}")
                            ca = prop.tile([N, HB], f32, tag=f"ca{h}",
                                           name=f"ca{h}{sfx}")
                            nc.scalar.activation(sa[:], pre[:, hs], AF.Sin,
                                                 bias=bsa[:])
                            nc.scalar.activation(ca[:], pre[:, hs], AF.Sin,
                                                 bias=bca[:])
                            t1 = prop.tile([N, HB], f32, tag=f"t1{h}",
                                           name=f"t1{h}{sfx}")
                            u1 = prop.tile([N, HB], f32, tag=f"u1{h}",
                                           name=f"u1{h}{sfx}")
                            nc.vector.tensor_mul(t1[:], sa[:], ca[:])
                            nc.vector.tensor_mul(u1[:], sa[:], sa[:])
                            c2 = prop.tile([N, HB], f32, tag=f"c2{h}",
                                           name=f"c2{h}{sfx}")
                            nc.scalar.activation(c2[:], u1[:], AF.Copy,
                                                 bias=1.0, scale=-2.0)
                            t2 = prop.tile([N, HB], f32, tag=f"t2{h}",
                                           name=f"t2{h}{sfx}")
                            u2 = prop.tile([N, HB], f32, tag=f"u2{h}",
                                           name=f"u2{h}{sfx}")
                            nc.vector.scalar_tensor_tensor(
                                t2[:], t1[:], 2.0, c2[:], ALU.mult, ALU.mult)
                            nc.vector.scalar_tensor_tensor(
                                u2[:], t1[:], 4.0, t1[:], ALU.mult, ALU.mult)
                            nc.vector.tensor_add(vboth[32:32 + N, hs],
                                                 t2[:], t2[:])
                            nc.scalar.activation(vboth[0:N, hs], u2[:],
                                                 AF.Copy, bias=1.0,
                                                 scale=-2.0)

                    # half-kron factors aT, bloT + bloTrep, per batch half
                    with tc.tile_pool(name="fps", bufs=1,
                                      space="PSUM") as f_ps:
                        for h in (0, 1):
                            hs = slice(h * HB, (h + 1) * HB)
                            for half, dst in ((0, aT), (1, bloT)):
                                Ms = []
                                for pair in range(3):
                                    Fp = [None, None]
                                    for k in range(2):
                                        w = half * 6 + pair * 2 + k
                                        Fp[k] = f_ps.tile(
                                            [64, HB], f32, tag=f"F{k}",
                                            name=f"F{k}_{h}{half}{pair}{sfx}")
                                        nc.tensor.matmul(
                                            Fp[k][:],
                                            sel_sb[:,
                                                   w * 64:(w + 1) * 64],
                                            vboth[:, hs],
                                            start=True, stop=True)
                                    fcp = prop.tile(
                                        [64, HB], f32, tag=f"fcp{h}",
                                        name=f"fcp_{h}{half}{pair}{sfx}")
                                    nc.scalar.activation(fcp[:], Fp[1][:],
                                                         AF.Copy)
                                    M = prop.tile(
                                        [64, HB], f32, tag=f"m{pair}{h}",
                                        name=f"M{pair}_{h}{half}{sfx}")
                                    nc.vector.tensor_mul(M[:], Fp[0][:],
                                                         fcp[:])
                                    Ms.append(M)
                                M01 = prop.tile([64, HB], f32,
                                                tag=f"fcp{h}",
                                                name=f"M01_{h}{half}{sfx}")
                                nc.gpsimd.tensor_mul(M01[:], Ms[0][:],
                                                     Ms[1][:])
                                nc.gpsimd.tensor_mul(dst[:, hs], M01[:],
                                                     Ms[2][:])
                            # bloTrep[p, b] = bloT[p & 63, b]
                            bt = f_ps.tile([128, HB], f32, tag="bt",
                                           name=f"bt{h}{sfx}")
                            nc.tensor.matmul(
                                bt[:],
                                rsel_sb[:, 32 * 128:33 * 128],
                                bloT[:, hs],
                                start=True, stop=True)
                            nc.scalar.activation(bloTrep[:, hs], bt[:],
                                                 AF.Copy)

                # ---------- s0 generation + main DR matmul, per batch half
                s0hi = [[s0p.tile([128, 2, HB], e4, tag=f"s0h{g_}_{h}",
                                  name=f"s0h{g_}_{h}{sfx}")
                         for g_ in range(16)] for h in (0, 1)]
                s0lo = [[s0p.tile([128, 2, HB], e4, tag=f"s0l{g_}_{h}",
                                  name=f"s0l{g_}_{h}{sfx}")
                         for g_ in range(16)] for h in (0, 1)]
                acc_pos = accp.tile([128, B_CORE], f32r, tag="accp",
                                    name="accp" + sfx)
                acc_neg = accp.tile([128, B_CORE], f32r, tag="accn",
                                    name="accn" + sfx)
                out_sb = accp.tile([1, B_CORE], f32, tag="osb",
                                   name="osb" + sfx)
                with (
                    tc.tile_pool(name="repps", bufs=2, space="PSUM") as rep_ps,
                    tc.tile_pool(name="mmps", bufs=3, space="PSUM") as mm_ps,
                    tc.tile_pool(name="finps", bufs=1, space="PSUM") as fin_ps,
                ):
                    # gen for both halves up front (h=1 vector work overlaps
                    # the h=0 matmul sweep)
                    for h in (0, 1):
                        hs = slice(h * HB, (h + 1) * HB)
                        for t in range(32):
                            g_, i_ = t // 2, t % 2
                            rp = rep_ps.tile([128, HB], f32, tag=f"rp{h}",
                                             name=f"rp{t}_{h}{sfx}")
                            nc.tensor.matmul(
                                rp[:],
                                rsel_sb[:, t * 128:(t + 1) * 128],
                                aT[:, hs],
                                start=True, stop=True)
                            pool_t = t % 8 < 5
                            seng = nc.gpsimd if pool_t else nc.vector
                            tmp = tmpp.tile([128, HB], bf16,
                                            tag=f"tmp{int(pool_t)}{h}")
                            nc.vector.tensor_mul(tmp[:], rp[:],
                                                 bloTrep[:, hs])
                            nc.scalar.activation(s0hi[h][g_][:, i_, :],
                                                 tmp[:], AF.Copy)
                            seng.tensor_sub(s0lo[h][g_][:, i_, :], tmp[:],
                                            s0hi[h][g_][:, i_, :])

                    # main DoubleRow sweeps, one per batch half
                    for h in (0, 1):
                        hs = slice(h * HB, (h + 1) * HB)
                        for jt in range(32):
                            utt = utp.tile([128, 16, 2, 128], e4,
                                           tag="uth",
                                           name=f"utt{jt}_{h}{sfx}")
                            (nc.sync if h == 0 else nc.gpsimd).dma_start(
                                utt[:], uth_d[jt])
                            sq = sqp.tile([128, HB], f32,
                                          tag=f"sq{jt % 2}",
                                          name=f"sq{jt}_{h}{sfx}")
                            ps = mm_ps.tile([128, HB], f32, tag="mm",
                                            name=f"mm{jt}_{h}{sfx}")
                            for kp in range(16):
                                nc.tensor.matmul(
                                    ps[:], utt[:, kp],
                                    s0hi[h][kp][:, :, :],
                                    start=(kp == 0), stop=False,
                                    perf_mode=DR)
                            for kp in range(16):
                                nc.tensor.matmul(
                                    ps[:], utt[:, kp],
                                    s0lo[h][kp][:, :, :],
                                    start=False, stop=(kp == 15),
                                    perf_mode=DR)
                            nc.scalar.activation(sq[:], ps[:], AF.Square)
                            acc = acc_pos if jt < 16 else acc_neg
                            eng = nc.vector if jt < 16 else nc.gpsimd
                            if jt % 16 == 0:
                                eng.tensor_copy(acc[:, hs], sq[:])
                            else:
                                eng.tensor_add(acc[:, hs], acc[:, hs],
                                               sq[:])

                    # final +/- partition reduction, scale, bias
                    for h in (0, 1):
                        hs = slice(h * HB, (h + 1) * HB)
                        fin = fin_ps.tile([1, HB], f32, tag="fin",
                                          name=f"fin{h}{sfx}")
                        nc.tensor.matmul(fin[:], ones,
                                         acc_pos[:, hs],
                                         start=True, stop=False)
                        nc.tensor.matmul(fin[:], mones,
                                         acc_neg[:, hs],
                                         start=False, stop=True)
                        nc.scalar.activation(
                            out_sb[:, hs], fin[:], AF.Identity,
                            bias=pb_sb[:], scale=float(1.0 / (S_U * S_S) ** 2))
                    nc.sync.dma_start(out_d[:], out_sb[:])

    nc.compile()
    return nc


# ------------------------------------------------------------------- entry
def _core_input_map(inputs, core):
    x = np.ascontiguousarray(np.asarray(inputs["x"], dtype=np.float32))
    wf_packed, bias_sa, bias_ca, sel, rsel, ut_h, ut_l, pb = _host_constants(
        inputs["pre_w"], inputs["pre_b"], inputs["weights"],
        inputs["post_w"], inputs["post_b"])
    return {
        "x": x[core * B_CORE:(core + 1) * B_CORE],
        "wf": wf_packed, "bsa": bias_sa, "bca": bias_ca,
        "sel": sel, "rsel": rsel, "uth": ut_h, "utl": ut_l, "pb": pb,
    }


def kernel(x, pre_w, pre_b, weights, post_w, post_b):
    from concourse import bass_utils

    inputs = {"x": x, "pre_w": pre_w, "pre_b": pre_b, "weights": weights,
              "post_w": post_w, "post_b": post_b}

    if "nc" not in _prog_cache:
        _prog_cache["nc"] = _build_program()
    nc = _prog_cache["nc"]

    in_maps = [_core_input_map(inputs, c) for c in range(N_CORES)]
    res = bass_utils.run_bass_kernel_spmd(nc, in_maps,
                                          core_ids=list(range(N_CORES)))
    out = np.concatenate([r["out"][0] for r in res.results])
    return out.reshape(BATCH, 1).astype(np.float32)


# revision 23
# speedup vs baseline: 1.4543x; 1.0940x over previous
"""Trainium2 Bass kernel for DressedQuantumCircuit (12 qubits, 6 layers).

Strategy (fp8 DoubleRow formulation):
  - Layer-1 RY gates fold into the input angles; the rest of the circuit is
    a fixed orthogonal U' in R^{4096x4096} precomputed on host from
    `weights`.  Readout contracts to out[b] = sum_j g_j (U'^T s0_b)_j^2
    + post_b, with g_j = sum_w post_w[w] (1-2 bit_w(j)).
  - sqrt|g_j| is folded into U' columns; columns are permuted so the first
    2048 have g>0 and the last 2048 g<0 (exactly balanced by j -> ~j
    symmetry), so the readout is (+1/-1) partition reductions.
  - Data-parallel over batch: 8192 samples -> 8 cores x 1024.
  - Per core: transpose x, pre-matmul -> quarter angles, ACT Sin + two
    double-angle steps -> per-wire (cos G, sin G); half-statevector
    Kronecker factors a[64] (wires 0-5) and blo[64] (wires 6-11) built via
    PE selection matmuls + elementwise product trees; s0[i,b] assembled as
    a replicated-row product, quantized to fp8 e4m3 as hi + exact-residual
    lo (two buffers -> effectively ~12-bit statevector).
  - Main contraction in fp8 with MatmulPerfMode.DoubleRow (256-deep
    contraction per instruction, 0.5 cycles/row): chains H^T s_hi +
    H^T s_lo (+ optional e5m2 residual-of-U chain, L_KP k-pairs).
  - Epilogue: ACT Square, +/- accumulate, (+1/-1)-ones matmul partition
    reduction, final scale + post_b bias.
"""

import hashlib

import numpy as np

N_QUBITS = 12
N_LAYERS = 6
D = 4096                 # 2**N_QUBITS
D_IN = 512
BATCH = 8192
N_CORES = 8
B_CORE = BATCH // N_CORES      # 1024

S_U = 64.0               # fp8 scale for U'
S_S = 16.0               # fp8 scale for s0 (folded into sel wire-11 block)
L_KP = 0                 # ut e5m2 residual chain coverage in k-pairs (0..16)

_prog_cache = {}
_ut_cache = {}
_const_cache = {}


# ----------------------------------------------------------------- host math
def _build_ut(weights):
    """Simulate CNOT-chain + layers 2..6 on the identity.  Row i of the
    result is circuit(e_i), i.e. result = U'^T: psi = ut.T @ s0."""
    key = hashlib.sha256(np.ascontiguousarray(weights)).hexdigest()
    if key in _ut_cache:
        return _ut_cache[key]
    N = N_QUBITS
    st = np.eye(D, dtype=np.float32)

    def ry_layer(st, thetas):
        for w in range(N):
            c = np.float32(np.cos(thetas[w] / 2))
            s = np.float32(np.sin(thetas[w] / 2))
            lo = 2 ** (N - 1 - w)
            sh = st.reshape(D, -1, 2, lo)
            a = sh[:, :, 0, :].copy()
            b = sh[:, :, 1, :]
            sh[:, :, 0, :] = c * a - s * b
            sh[:, :, 1, :] = s * a + c * b
        return st

    def cnot_chain(st):
        for w in range(N - 1):
            lt = 2 ** (N - 2 - w)
            sh = st.reshape(D, -1, 2, 1, 2, lt)
            a = sh[:, :, 1, :, 0, :].copy()
            sh[:, :, 1, :, 0, :] = sh[:, :, 1, :, 1, :]
            sh[:, :, 1, :, 1, :] = a
        return st

    wts = np.asarray(weights, dtype=np.float64)
    st = cnot_chain(st)
    for layer in range(1, N_LAYERS):
        st = ry_layer(st, wts[layer])
        st = cnot_chain(st)
    _ut_cache.clear()
    _ut_cache[key] = st
    return st


def _host_constants(pre_w, pre_b, weights, post_w, post_b):
    import ml_dtypes

    key = hashlib.sha256(
        b"".join(np.ascontiguousarray(np.asarray(a, np.float64))
                 for a in (pre_w, pre_b, weights, post_w, post_b))
    ).hexdigest() + f"_{L_KP}"
    if key in _const_cache:
        return _const_cache[key]

    N = N_QUBITS
    wts = np.asarray(weights, dtype=np.float64)
    # quarter angle: a = x @ (pre_w.T * pi/16) + bias_a, full angle G = 4a,
    # v0 = cos(G), v1 = sin(G)
    wf = (np.asarray(pre_w, dtype=np.float64).T * (np.pi / 16.0))  # [512, 12]
    bias_a = (np.asarray(pre_b, dtype=np.float64) * (np.pi / 4.0)
              + wts[0] / 2.0 + np.pi / 4.0) / 4.0
    wf_packed = np.ascontiguousarray(
        wf.reshape(4, 128, N).transpose(1, 0, 2).reshape(128, 4 * N)
    ).astype(np.float32)
    bias_sa = bias_a.reshape(N, 1).astype(np.float32)
    bias_ca = (bias_a + np.pi / 2.0).reshape(N, 1).astype(np.float32)

    # readout vector g, sign-sorted column permutation
    j = np.arange(D)
    g = np.zeros(D, dtype=np.float64)
    for w in range(N):
        g += float(np.asarray(post_w).reshape(-1)[w]) * (
            1.0 - 2.0 * ((j >> (N - 1 - w)) & 1))
    pos = np.where(g > 0)[0]
    neg = np.where(g <= 0)[0]
    assert len(pos) == D // 2 and len(neg) == D // 2, (len(pos), len(neg))
    perm = np.concatenate([pos, neg])
    sg = np.sqrt(np.abs(g[perm])).astype(np.float32)

    ut = _build_ut(weights)                       # [4096 i, 4096 j]
    utg = ut[:, perm].astype(np.float32) * sg[None, :]
    Hq = (utg * np.float32(S_U)).astype(ml_dtypes.float8_e4m3)
    H32 = Hq.astype(np.float32)
    Rq = (utg * np.float32(S_U) - H32).astype(ml_dtypes.float8_e5m2)

    def pack(M):
        # [4096 k, 4096 j] -> [32 j, 128 p, 16 kp, 2 i, 128 jj]
        # k = kp*256 + i*128 + p
        return np.ascontiguousarray(
            M.reshape(16, 2, 128, 32, 128).transpose(3, 2, 0, 1, 4))

    ut_h = pack(Hq)
    ut_l = pack(Rq)[:, :, :max(L_KP, 1), :, :].copy()

    # sel [44, 12*64]: F_w[q, b] = v_{w, bit}(b); v0 rows 0..11, v1 rows
    # 32..43 (32-aligned for DVE writes); wire-11 block carries S_S.
    sel = np.zeros((44, 12 * 64), np.float32)
    q64 = np.arange(64)
    for w in range(12):
        bit = (q64 >> (5 - (w % 6))) & 1
        sel[w, w * 64 + q64[bit == 0]] = 1.0
        sel[32 + w, w * 64 + q64[bit == 1]] = 1.0
    sel[:, 11 * 64:12 * 64] *= np.float32(S_S)

    # rsel [64, 33*128]: blocks t=0..31: rep_t[p,b] = aT[2t + (p>>6), b];
    # block 32: bloTrep[p,b] = bloT[p & 63, b]
    rsel = np.zeros((64, 33 * 128), np.float32)
    p128 = np.arange(128)
    for t in range(32):
        rsel[2 * t + (p128 >> 6), t * 128 + p128] = 1.0
    rsel[(p128 & 63), 32 * 128 + p128] = 1.0

    pb = np.asarray(post_b, dtype=np.float32).reshape(1, 1)
    out = (wf_packed, bias_sa, bias_ca, sel, rsel, ut_h, ut_l, pb)
    _const_cache.clear()
    _const_cache[key] = out
    return out


# ------------------------------------------------------------- device program
def _build_program(reps=1, l_kp=L_KP):
    import concourse.bass as bass
    import concourse.mybir as mybir
    import concourse.tile as tile
    from concourse import bacc
    from concourse.masks import make_identity

    f32 = mybir.dt.float32
    f32r = mybir.dt.float32r
    e4 = mybir.dt.float8e4
    bf16 = mybir.dt.bfloat16
    e5 = mybir.dt.float8e5
    AF = mybir.ActivationFunctionType
    ALU = mybir.AluOpType
    DR = mybir.MatmulPerfMode.DoubleRow
    N = N_QUBITS

    nc = bacc.Bacc("TRN2", target_bir_lowering=False, debug=False,
                   num_devices=N_CORES)
    x_d = nc.dram_tensor("x", [B_CORE, D_IN], f32r, kind="ExternalInput").ap()
    wf_d = nc.dram_tensor("wf", [128, 4 * N], f32r, kind="ExternalInput").ap()
    bsa_d = nc.dram_tensor("bsa", [N, 1], f32, kind="ExternalInput").ap()
    bca_d = nc.dram_tensor("bca", [N, 1], f32, kind="ExternalInput").ap()
    sel_d = nc.dram_tensor("sel", [44, 12 * 64], f32r,
                           kind="ExternalInput").ap()
    rsel_d = nc.dram_tensor("rsel", [64, 33 * 128], f32r,
                            kind="ExternalInput").ap()
    uth_d = nc.dram_tensor("uth", [32, 128, 16, 2, 128], e4,
                           kind="ExternalInput").ap()
    utl_d = nc.dram_tensor("utl", [32, 128, max(l_kp, 1), 2, 128], e5,
                           kind="ExternalInput").ap()
    pb_d = nc.dram_tensor("pb", [1, 1], f32, kind="ExternalInput").ap()
    out_d = nc.dram_tensor("out", [1, B_CORE], f32, kind="ExternalOutput").ap()

    HB = 512                       # half-batch width

    with tile.TileContext(nc) as tc:
        with (
            tc.tile_pool(name="const", bufs=1) as constp,
            tc.tile_pool(name="ab", bufs=1) as abp,
            tc.tile_pool(name="s0", bufs=1) as s0p,
            tc.tile_pool(name="tmp", bufs=2) as tmpp,
            tc.tile_pool(name="ut", bufs=3) as utp,
            tc.tile_pool(name="acc", bufs=1) as accp,
            tc.tile_pool(name="sq", bufs=1) as sqp,
        ):
            wf_sb = constp.tile([128, 4 * N], f32r)
            nc.gpsimd.dma_start(wf_sb[:], wf_d[:])
            bsa = constp.tile([N, 1], f32)
            nc.gpsimd.dma_start(bsa[:], bsa_d[:])
            bca = constp.tile([N, 1], f32)
            nc.gpsimd.dma_start(bca[:], bca_d[:])
            sel_sb = constp.tile([44, 12 * 64], f32r)
            nc.scalar.dma_start(sel_sb[:], sel_d[:])
            rsel_sb = constp.tile([64, 33 * 128], f32r)
            nc.scalar.dma_start(rsel_sb[:], rsel_d[:])
            pb_sb = constp.tile([1, 1], f32)
            nc.gpsimd.dma_start(pb_sb[:], pb_d[:])
            pm0 = constp.tile([128, 2], f32)
            nc.gpsimd.memset(pm0[:, 0:1], 1.0)
            nc.gpsimd.memset(pm0[:, 1:2], -1.0)
            pm1 = constp.tile([128, 2], f32r)
            nc.vector.tensor_copy(pm1[:], pm0[:])
            ones = pm1[:, 0:1]
            mones = pm1[:, 1:2]

            for rep in range(reps):
                sfx = f"_{rep}"
                aT = abp.tile([64, B_CORE], f32r, tag="aT", name="aT" + sfx)
                bloT = abp.tile([64, B_CORE], f32r, tag="bloT",
                                name="bloT" + sfx)
                bloTrep = abp.tile([128, B_CORE], f32r, tag="bTr",
                                   name="bTr" + sfx)
                # ---------- prologue: x -> angles -> (cos G, sin G) -> a, blo
                # (everything after the pre-matmul is split by batch half so
                # the h=0 pipeline races ahead)
                with tc.tile_pool(name="pro", bufs=1) as prop:
                    ident0 = prop.tile([128, 128], f32, tag="id0",
                                       name="ident0" + sfx)
                    make_identity(nc, ident0[:])
                    ident = prop.tile([128, 128], f32r, tag="ident",
                                      name="ident" + sfx)
                    nc.vector.tensor_copy(ident[:], ident0[:])
                    xTh = [prop.tile([128, 4, HB], f32r, tag=f"xT{h}",
                                     name=f"xT{h}{sfx}") for h in (0, 1)]
                    vboth = prop.tile([44, B_CORE], f32r, tag="vb",
                                      name="vb" + sfx)
                    nc.scalar.activation(vboth[0:32, :],
                                         rsel_sb[0:32, 0:B_CORE],
                                         AF.Copy, scale=0.0)
                    with (
                        tc.tile_pool(name="trps", bufs=2,
                                     space="PSUM") as tr_ps,
                        tc.tile_pool(name="preps", bufs=1,
                                     space="PSUM") as pre_ps,
                    ):
                        for cc in range(8):
                            xt = prop.tile([128, D_IN], f32r,
                                           tag=f"xt{cc % 2}",
                                           name=f"xt{cc}{sfx}")
                            (nc.sync if cc % 2 == 0 else nc.gpsimd
                             ).dma_start(
                                xt[:], x_d[cc * 128:(cc + 1) * 128, :])
                            tp = tr_ps.tile([128, 4, 128], f32r, tag="tp")
                            for kf in range(4):
                                nc.tensor.transpose(
                                    tp[:, kf, :],
                                    xt[:, kf * 128:(kf + 1) * 128], ident[:])
                            dst = xTh[cc // 4][
                                :, :, (cc % 4) * 128:(cc % 4 + 1) * 128]
                            if cc % 2:
                                nc.vector.tensor_copy(dst, tp[:])
                            else:
                                nc.scalar.activation(dst, tp[:], AF.Copy)
                        # pre-matmul: quarter angles [12, 1024]
                        pre = pre_ps.tile([N, B_CORE], f32, tag="pre")
                        for h in (0, 1):
                            hs = slice(h * HB, (h + 1) * HB)
                            for kf in range(4):
                                nc.tensor.matmul(
                                    pre[:, hs],
                                    wf_sb[:, kf * N:(kf + 1) * N],
                                    xTh[h][:, kf, :],
                                    start=(kf == 0), stop=(kf == 3))
                        # per-half double-angle chain
                        for h in (0, 1):
                            hs = slice(h * HB, (h + 1) * HB)
                            sa = prop.tile([N, HB], f32, tag=f"sa{h}",
                                           name=f"sa{h}{sfx}")
                            ca = prop.tile([N, HB], f32, tag=f"ca{h}",
                                           name=f"ca{h}{sfx}")
                            nc.scalar.activation(sa[:], pre[:, hs], AF.Sin,
                                                 bias=bsa[:])
                            nc.scalar.activation(ca[:], pre[:, hs], AF.Sin,
                                                 bias=bca[:])
                            t1 = prop.tile([N, HB], f32, tag=f"t1{h}",
                                           name=f"t1{h}{sfx}")
                            u1 = prop.tile([N, HB], f32, tag=f"u1{h}",
                                           name=f"u1{h}{sfx}")
                            nc.vector.tensor_mul(t1[:], sa[:], ca[:])
                            nc.vector.tensor_mul(u1[:], sa[:], sa[:])
                            c2 = prop.tile([N, HB], f32, tag=f"c2{h}",
                                           name=f"c2{h}{sfx}")
                            nc.scalar.activation(c2[:], u1[:], AF.Copy,
                                                 bias=1.0, scale=-2.0)
                            t2 = prop.tile([N, HB], f32, tag=f"t2{h}",
                                           name=f"t2{h}{sfx}")
                            u2 = prop.tile([N, HB], f32, tag=f"u2{h}",
                                           name=f"u2{h}{sfx}")
                            nc.vector.scalar_tensor_tensor(
                                t2[:], t1[:], 2.0, c2[:], ALU.mult, ALU.mult)
                            nc.vector.scalar_tensor_tensor(
                                u2[:], t1[:], 4.0, t1[:], ALU.mult, ALU.mult)
                            nc.vector.tensor_add(vboth[32:32 + N, hs],
                                                 t2[:], t2[:])
                            nc.scalar.activation(vboth[0:N, hs], u2[:],
                                                 AF.Copy, bias=1.0,
                                                 scale=-2.0)

                    # half-kron factors aT, bloT + bloTrep, per batch half
                    with tc.tile_pool(name="fps", bufs=1,
                                      space="PSUM") as f_ps:
                        for h in (0, 1):
                            hs = slice(h * HB, (h + 1) * HB)
                            for half, dst in ((0, aT), (1, bloT)):
                                Ms = []
                                for pair in range(3):
                                    Fp = [None, None]
                                    for k in range(2):
                                        w = half * 6 + pair * 2 + k
                                        Fp[k] = f_ps.tile(
                                            [64, HB], f32, tag=f"F{k}",
                                            name=f"F{k}_{h}{half}{pair}{sfx}")
                                        nc.tensor.matmul(
                                            Fp[k][:],
                                            sel_sb[:,
                                                   w * 64:(w + 1) * 64],
                                            vboth[:, hs],
                                            start=True, stop=True)
                                    fcp = prop.tile(
                                        [64, HB], f32, tag=f"fcp{h}",
                                        name=f"fcp_{h}{half}{pair}{sfx}")
                                    nc.scalar.activation(fcp[:], Fp[1][:],
                                                         AF.Copy)
                                    M = prop.tile(
                                        [64, HB], f32, tag=f"m{pair}{h}",
                                        name=f"M{pair}_{h}{half}{sfx}")
                                    nc.vector.tensor_mul(M[:], Fp[0][:],
                                                         fcp[:])
                                    Ms.append(M)
                                M01 = prop.tile([64, HB], f32,
                                                tag=f"fcp{h}",
                                                name=f"M01_{h}{half}{sfx}")
                                nc.gpsimd.tensor_mul(M01[:], Ms[0][:],
                                                     Ms[1][:])
                                nc.gpsimd.tensor_mul(dst[:, hs], M01[:],
                                                     Ms[2][:])
                            # bloTrep[p, b] = bloT[p & 63, b]
                            bt = f_ps.tile([128, HB], f32, tag="bt",
                                           name=f"bt{h}{sfx}")
                            nc.tensor.matmul(
                                bt[:],
                                rsel_sb[:, 32 * 128:33 * 128],
                                bloT[:, hs],
                                start=True, stop=True)
                            nc.scalar.activation(bloTrep[:, hs], bt[:],
                                                 AF.Copy)

                # ---------- s0 generation + main DR matmul, per batch half
                s0hi = [[s0p.tile([128, 2, HB], e4, tag=f"s0h{g_}_{h}",
                                  name=f"s0h{g_}_{h}{sfx}")
                         for g_ in range(16)] for h in (0, 1)]
                s0lo = [[s0p.tile([128, 2, HB], e4, tag=f"s0l{g_}_{h}",
                                  name=f"s0l{g_}_{h}{sfx}")
                         for g_ in range(16)] for h in (0, 1)]
                acc_pos = accp.tile([128, B_CORE], f32r, tag="accp",
                                    name="accp" + sfx)
                acc_neg = accp.tile([128, B_CORE], f32r, tag="accn",
                                    name="accn" + sfx)
                out_sb = accp.tile([1, B_CORE], f32, tag="osb",
                                   name="osb" + sfx)
                with (
                    tc.tile_pool(name="repps", bufs=2, space="PSUM") as rep_ps,
                    tc.tile_pool(name="mmps", bufs=3, space="PSUM") as mm_ps,
                    tc.tile_pool(name="finps", bufs=1, space="PSUM") as fin_ps,
                ):
                    # gen for both halves up front (h=1 vector work overlaps
                    # the h=0 matmul sweep)
                    for h in (0, 1):
                        hs = slice(h * HB, (h + 1) * HB)
                        for t in range(32):
                            g_, i_ = t // 2, t % 2
                            rp = rep_ps.tile([128, HB], f32, tag=f"rp{h}",
                                             name=f"rp{t}_{h}{sfx}")
                            nc.tensor.matmul(
                                rp[:],
                                rsel_sb[:, t * 128:(t + 1) * 128],
                                aT[:, hs],
                                start=True, stop=True)
                            pool_t = t % 8 < 5
                            seng = nc.gpsimd if pool_t else nc.vector
                            tmp = tmpp.tile([128, HB], bf16,
                                            tag=f"tmp{int(pool_t)}{h}")
                            nc.vector.tensor_mul(tmp[:], rp[:],
                                                 bloTrep[:, hs])
                            nc.scalar.activation(s0hi[h][g_][:, i_, :],
                                                 tmp[:], AF.Copy)
                            seng.tensor_sub(s0lo[h][g_][:, i_, :], tmp[:],
                                            s0hi[h][g_][:, i_, :])

                    # main DoubleRow sweeps, one per batch half
                    for h in (0, 1):
                        hs = slice(h * HB, (h + 1) * HB)
                        for jt in range(32):
                            utt = utp.tile([128, 16, 2, 128], e4,
                                           tag="uth",
                                           name=f"utt{jt}_{h}{sfx}")
                            (nc.sync if h == 0 else nc.gpsimd).dma_start(
                                utt[:], uth_d[jt])
                            sq = sqp.tile([128, HB], f32,
                                          tag=f"sq{jt % 2}",
                                          name=f"sq{jt}_{h}{sfx}")
                            ps = mm_ps.tile([128, HB], f32, tag="mm",
                                            name=f"mm{jt}_{h}{sfx}")
                            for kp in range(16):
                                nc.tensor.matmul(
                                    ps[:], utt[:, kp],
                                    s0hi[h][kp][:, :, :],
                                    start=(kp == 0), stop=False,
                                    perf_mode=DR)
                            for kp in range(16):
                                nc.tensor.matmul(
                                    ps[:], utt[:, kp],
                                    s0lo[h][kp][:, :, :],
                                    start=False, stop=(kp == 15),
                                    perf_mode=DR)
                            nc.scalar.activation(sq[:], ps[:], AF.Square)
                            acc = acc_pos if jt < 16 else acc_neg
                            eng = nc.vector if jt < 16 else nc.gpsimd
                            if jt % 16 == 0:
                                eng.tensor_copy(acc[:, hs], sq[:])
                            else:
                                eng.tensor_add(acc[:, hs], acc[:, hs],
                                               sq[:])
                        # per-half +/- partition reduction, scale, bias
                        fin = fin_ps.tile([1, HB], f32, tag="fin",
                                          name=f"fin{h}{sfx}")
                        nc.tensor.matmul(fin[:], ones,
                                         acc_pos[:, hs],
                                         start=True, stop=False)
                        nc.tensor.matmul(fin[:], mones,
                                         acc_neg[:, hs],
                                         start=False, stop=True)
                        nc.scalar.activation(
                            out_sb[:, hs], fin[:], AF.Identity,
                            bias=pb_sb[:], scale=float(1.0 / (S_U * S_S) ** 2))
                        nc.sync.dma_start(out_d[0:1, hs], out_sb[:, hs])

    nc.compile()
    return nc


# ------------------------------------------------------------------- entry
def _core_input_map(inputs, core):
    x = np.ascontiguousarray(np.asarray(inputs["x"], dtype=np.float32))
    wf_packed, bias_sa, bias_ca, sel, rsel, ut_h, ut_l, pb = _host_constants(
        inputs["pre_w"], inputs["pre_b"], inputs["weights"],
        inputs["post_w"], inputs["post_b"])
    return {
        "x": x[core * B_CORE:(core + 1) * B_CORE],
        "wf": wf_packed, "bsa": bias_sa, "bca": bias_ca,
        "sel": sel, "rsel": rsel, "uth": ut_h, "utl": ut_l, "pb": pb,
    }


def kernel(x, pre_w, pre_b, weights, post_w, post_b):
    from concourse import bass_utils

    inputs = {"x": x, "pre_w": pre_w, "pre_b": pre_b, "weights": weights,
              "post_w": post_w, "post_b": post_b}

    if "nc" not in _prog_cache:
        _prog_cache["nc"] = _build_program()
    nc = _prog_cache["nc"]

    in_maps = [_core_input_map(inputs, c) for c in range(N_CORES)]
    res = bass_utils.run_bass_kernel_spmd(nc, in_maps,
                                          core_ids=list(range(N_CORES)))
    out = np.concatenate([r["out"][0] for r in res.results])
    return out.reshape(BATCH, 1).astype(np.float32)
